# revision 1
# baseline (speedup 1.0000x reference)
"""Trainium2 Bass kernel for nn_KStackModel (sparse_attention).

Strategy: data-parallel over batch (8 batches -> 8 cores, no collectives).
Per core, the whole layer runs in a feature-major activation layout so no
on-device fp32 transposes are needed:

  h [t,d] (token-major) + hT [d,t] (host-transposed) are both DMA'd in.
  rms1 stats via ACT Square+accum_out (free-dim reduce on token-major h).
  hn = h * rstd (token-major, stationary operand for base matmul).
  hnT = hT * rstd_rep (rstd broadcast via tiny PE transpose + outer product).
  xv^T[r,t] = v_eff.T @ hnT  (PE, rank-major).
  mixed^T = decay-scan over t: one DVE tensor_tensor_scan (state=g*state+xv).
  out^T[d,t] = hn-blocks.T @ KT (upper-tri blocks only) + u_effT.T @ mixed^T,
               accumulated in one PSUM group.
  h1^T = pw-blocks.T @ out^T + proj_b + hT   (feature-major, bias per-partition)
  rms2 via ACT Square + PE ones-column reduction (partition-dim reduce).
  g^T = gelu(up-blocks.T @ h2^T + up_b);  y^T = dw-blocks.T @ g^T + down_b + h1^T
  y^T is DMA'd out feature-major; the host transposes it back.

All weight reshapes/folds (norm scales, gate, alpha) are exact host-side
algebra; everything touching h runs on device.
"""
import numpy as np
from contextlib import ExitStack

import concourse.bass as bass
import concourse.bacc as bacc
import concourse.tile as tile
from concourse import mybir
from concourse.bass_utils import run_bass_kernel_spmd

B, W, D, R, F = 8, 1024, 1024, 32, 2048
NT, ND, NF = W // 128, D // 128, F // 128   # 8, 8, 16
FP = mybir.dt.float32
GAMMA_MIN, GAMMA_MAX = 0.15, 1.0
AF = mybir.ActivationFunctionType
ALU = mybir.AluOpType


def _emit(ctx, tc, a):
    nc = tc.nc

    big = ctx.enter_context(tc.tile_pool(name="big", bufs=24))
    wst = ctx.enter_context(tc.tile_pool(name="wst", bufs=2))
    hst = ctx.enter_context(tc.tile_pool(name="hst", bufs=2))
    kst = ctx.enter_context(tc.tile_pool(name="kst", bufs=8))
    sqs = ctx.enter_context(tc.tile_pool(name="sqs", bufs=2))
    con = ctx.enter_context(tc.tile_pool(name="con", bufs=1))
    rep = ctx.enter_context(tc.tile_pool(name="rep", bufs=1))
    sml = ctx.enter_context(tc.tile_pool(name="sml", bufs=26))
    yst = ctx.enter_context(tc.tile_pool(name="yst", bufs=2))
    pmm = ctx.enter_context(tc.tile_pool(name="pmm", bufs=4, space="PSUM"))
    psm = ctx.enter_context(tc.tile_pool(name="psm", bufs=1, space="PSUM"))

    # ---- constants ----
    zeros_c = con.tile([128, 1], FP, tag="zeros_c")
    nc.vector.memset(zeros_c[:], 0.0)
    nc.const_aps.aps[(FP, 0.0)] = zeros_c[:]
    eps_c = con.tile([128, 1], FP, tag="eps_c")
    nc.vector.memset(eps_c[:], 1e-8)
    nc.const_aps.aps[(FP, 1e-8)] = eps_c[:]
    ident = con.tile([128, 128], FP, tag="ident")
    nc.sync.dma_start(ident[:], a["ident"][:, :])
    ones_row = con.tile([1, 128], FP, tag="ones_row")
    nc.vector.memset(ones_row[:], 1.0)
    ones_col = con.tile([128, 1], FP, tag="ones_col")
    nc.vector.memset(ones_col[:], 1.0)
    u_sb = con.tile([R, D], FP, tag="u_sb")
    nc.sync.dma_start(u_sb[:], a["u_effT"][:, :])
    gam_sb = con.tile([R, W], FP, tag="gam_sb")
    nc.sync.dma_start(gam_sb[:], a["gamma_t"][:, :])
    projb = con.tile([128, ND], FP, tag="projb")
    nc.sync.dma_start(projb[:], a["projb"][:, :])
    downb = con.tile([128, ND], FP, tag="downb")
    nc.sync.dma_start(downb[:], a["downb"][:, :])
    upb = con.tile([128, NF], FP, tag="upb")
    nc.sync.dma_start(upb[:], a["upb"][:, :])
    v_sb = []
    for dj in range(ND):
        t = con.tile([128, R], FP, tag=f"v{dj}")
        nc.sync.dma_start(t[:], a["v_eff"][dj * 128:(dj + 1) * 128, :])
        v_sb.append(t)

    # ---- P1-P3: load h, rms1 stats, hn ----
    htok, rstd = [], []
    for ti in range(NT):
        ht = big.tile([128, D], FP, tag="big")
        nc.sync.dma_start(ht[:], a["h_tok"][ti * 128:(ti + 1) * 128, :])
        htok.append(ht)
    for ti in range(NT):
        sq = sqs.tile([128, D], FP, tag="sqs")
        ssq = sml.tile([128, 1], FP, tag="sml")
        nc.scalar.activation(sq[:], htok[ti][:], AF.Square, accum_out=ssq[:])
        std = sml.tile([128, 1], FP, tag="sml")
        nc.scalar.activation(std[:], ssq[:], AF.Sqrt, bias=1e-8, scale=1.0 / D)
        rs = sml.tile([128, 1], FP, tag="sml")
        nc.vector.reciprocal(rs[:], std[:])
        rstd.append(rs)
    hn = []
    for ti in range(NT):
        t = big.tile([128, D], FP, tag="big")
        nc.scalar.activation(t[:], htok[ti][:], AF.Copy, scale=rstd[ti][:])
        hn.append(t)

    # ---- P4: rstd_row (PE transpose of [128,1] cols) -> rstd_rep [128, W] ----
    p_row = psm.tile([1, W], FP, tag="psm_row", bufs=1)
    for ti in range(NT):
        nc.tensor.transpose(p_row[0:1, ti * 128:(ti + 1) * 128], rstd[ti][:], ident[:])
    rstd_row = sml.tile([1, W], FP, tag="sml_row", bufs=3)
    nc.vector.tensor_copy(rstd_row[:], p_row[:])
    rep1 = rep.tile([128, W], FP, tag="rep")
    for tcc in range(2):
        p_rep = pmm.tile([128, 512], FP, tag="pmm")
        nc.tensor.matmul(p_rep[:], ones_row[:], rstd_row[0:1, tcc * 512:(tcc + 1) * 512],
                         start=True, stop=True)
        nc.vector.tensor_copy(rep1[:, tcc * 512:(tcc + 1) * 512], p_rep[:])

    # ---- P5: hnT = hT * rstd_rep ----
    hnT = []
    for dj in range(ND):
        hT_t = hst.tile([128, W], FP, tag="hst")
        nc.sync.dma_start(hT_t[:], a["h_T"][dj * 128:(dj + 1) * 128, :])
        t = big.tile([128, W], FP, tag="big")
        nc.vector.tensor_mul(t[:], hT_t[:], rep1[:])
        hnT.append(t)

    # ---- P6: xv^T [R, W] ----
    xvT = con.tile([R, W], FP, tag="xvT")
    for tcc in range(2):
        pxv = psm.tile([R, 512], FP, tag="psm_xv", bufs=1)
        for dj in range(ND):
            nc.tensor.matmul(pxv[:], v_sb[dj][:], hnT[dj][:, tcc * 512:(tcc + 1) * 512],
                             start=(dj == 0), stop=(dj == ND - 1))
        nc.vector.tensor_copy(xvT[:, tcc * 512:(tcc + 1) * 512], pxv[:])

    # ---- P7: decay scan ----
    mixedT = con.tile([R, W], FP, tag="mixedT")
    nc.vector.tensor_tensor_scan(mixedT[:], gam_sb[:], xvT[:], 0.0, ALU.mult, ALU.add)

    # ---- P8: out^T = base^T + lr^T ----
    outT = [big.tile([128, W], FP, tag="big", name=f"outT{dj}") for dj in range(ND)]
    for tcc in range(2):
        sjs = list(range(4)) if tcc == 0 else list(range(8))
        kts = {}
        for sj in sjs:
            kt = kst.tile([128, 512], FP, tag="kst")
            nc.sync.dma_start(kt[:], a["KT"][sj * 128:(sj + 1) * 128,
                                             tcc * 512:(tcc + 1) * 512])
            kts[sj] = kt
        for dj in range(ND):
            po = pmm.tile([128, 512], FP, tag="pmm")
            for i, sj in enumerate(sjs):
                nc.tensor.matmul(po[:], hn[sj][:, dj * 128:(dj + 1) * 128], kts[sj][:],
                                 start=(i == 0), stop=False)
            nc.tensor.matmul(po[:], u_sb[:, dj * 128:(dj + 1) * 128],
                             mixedT[:, tcc * 512:(tcc + 1) * 512],
                             start=False, stop=True)
            nc.vector.tensor_copy(outT[dj][:, tcc * 512:(tcc + 1) * 512], po[:])

    # ---- P9: h1^T = pw.T @ out^T + proj_b + hT ----
    h1T = []
    for dj2 in range(ND):
        pw_t = wst.tile([128, D], FP, tag="wst")
        nc.sync.dma_start(pw_t[:], a["pw"][dj2, :, :])
        hT_t = hst.tile([128, W], FP, tag="hst")
        nc.sync.dma_start(hT_t[:], a["h_T"][dj2 * 128:(dj2 + 1) * 128, :])
        h1 = big.tile([128, W], FP, tag="big")
        for tcc in range(2):
            ph = pmm.tile([128, 512], FP, tag="pmm")
            for dj in range(ND):
                nc.tensor.matmul(ph[:], pw_t[:, dj * 128:(dj + 1) * 128],
                                 outT[dj][:, tcc * 512:(tcc + 1) * 512],
                                 start=(dj == 0), stop=(dj == ND - 1))
            sl = slice(tcc * 512, (tcc + 1) * 512)
            nc.scalar.activation(h1[:, sl], ph[:], AF.Identity,
                                 bias=projb[:, dj2:dj2 + 1], scale=1.0)
            nc.vector.tensor_add(h1[:, sl], h1[:, sl], hT_t[:, sl])
        h1T.append(h1)

    # ---- P10: rms2 (feature-major): ssq over partitions via PE ones-col ----
    p_ssq = psm.tile([1, W], FP, tag="psm_row", bufs=1)
    for dj2 in range(ND):
        sq = sqs.tile([128, W], FP, tag="sqs")
        nc.scalar.activation(sq[:], h1T[dj2][:], AF.Square)
        for tcc in range(2):
            nc.tensor.matmul(p_ssq[0:1, tcc * 512:(tcc + 1) * 512], ones_col[:],
                             sq[:, tcc * 512:(tcc + 1) * 512],
                             start=(dj2 == 0), stop=(dj2 == ND - 1))
    std2 = sml.tile([1, W], FP, tag="sml_row", bufs=3)
    nc.scalar.activation(std2[:], p_ssq[:], AF.Sqrt, bias=1e-8, scale=1.0 / D)
    rstd2 = sml.tile([1, W], FP, tag="sml_row", bufs=3)
    nc.vector.reciprocal(rstd2[:], std2[:])
    rep2 = rep.tile([128, W], FP, tag="rep")
    for tcc in range(2):
        p_rep = pmm.tile([128, 512], FP, tag="pmm")
        nc.tensor.matmul(p_rep[:], ones_row[:], rstd2[0:1, tcc * 512:(tcc + 1) * 512],
                         start=True, stop=True)
        nc.vector.tensor_copy(rep2[:, tcc * 512:(tcc + 1) * 512], p_rep[:])

    # ---- P11: g^T = gelu((up.T @ h1^T) * rstd2[t] + up_b) ----
    # rstd2 column-scaling commutes through the d-contraction, so h2^T is
    # never materialized: scale the PSUM in place at eviction instead.
    gT = []
    for fi in range(NF):
        up_t = wst.tile([128, D], FP, tag="wst")
        nc.sync.dma_start(up_t[:], a["up"][fi, :, :])
        g = big.tile([128, W], FP, tag="big")
        for tcc in range(2):
            pg = pmm.tile([128, 512], FP, tag="pmm")
            for dj in range(ND):
                nc.tensor.matmul(pg[:], up_t[:, dj * 128:(dj + 1) * 128],
                                 h1T[dj][:, tcc * 512:(tcc + 1) * 512],
                                 start=(dj == 0), stop=(dj == ND - 1))
            nc.vector.tensor_mul(pg[:], pg[:], rep2[:, tcc * 512:(tcc + 1) * 512])
            nc.scalar.activation(g[:, tcc * 512:(tcc + 1) * 512], pg[:],
                                 AF.Gelu_apprx_tanh, bias=upb[:, fi:fi + 1], scale=1.0)
        gT.append(g)

    # ---- P12: y^T = dw.T @ g^T + down_b + h1^T ; DMA out ----
    for dj2 in range(ND):
        dw_t = wst.tile([128, F], FP, tag="wst")
        nc.sync.dma_start(dw_t[:], a["dw"][dj2, :, :])
        for tcc in range(2):
            py = pmm.tile([128, 512], FP, tag="pmm")
            for fi in range(NF):
                nc.tensor.matmul(py[:], dw_t[:, fi * 128:(fi + 1) * 128],
                                 gT[fi][:, tcc * 512:(tcc + 1) * 512],
                                 start=(fi == 0), stop=(fi == NF - 1))
            y = yst.tile([128, 512], FP, tag="yst")
            nc.scalar.activation(y[:], py[:], AF.Identity,
                                 bias=downb[:, dj2:dj2 + 1], scale=1.0)
            sl = slice(tcc * 512, (tcc + 1) * 512)
            nc.vector.tensor_add(y[:], y[:], h1T[dj2][:, sl])
            nc.sync.dma_start(a["yT"][dj2 * 128:(dj2 + 1) * 128, sl], y[:])


_NC_CACHE = {}


def _build():
    if "nc" in _NC_CACHE:
        return _NC_CACHE["nc"]
    nc = bacc.Bacc("TRN2", target_bir_lowering=False, debug=False)

    def P(name, shape, out=False):
        return nc.declare_dram_parameter(name, list(shape), FP, isOutput=out)

    a = dict(
        h_tok=P("h_tok", (W, D)),
        h_T=P("h_T", (D, W)),
        KT=P("KT", (W, W)),
        v_eff=P("v_eff", (D, R)),
        u_effT=P("u_effT", (R, D)),
        pw=P("pw", (ND, 128, D)),
        up=P("up", (NF, 128, D)),
        dw=P("dw", (ND, 128, F)),
        gamma_t=P("gamma_t", (R, W)),
        projb=P("projb", (128, ND)),
        downb=P("downb", (128, ND)),
        upb=P("upb", (128, NF)),
        ident=P("ident", (128, 128)),
        yT=P("yT", (D, W), out=True),
    )
    with ExitStack() as ctx:
        tcx = ctx.enter_context(tile.TileContext(nc))
        _emit(ctx, tcx, a)
    nc.finalize()
    _NC_CACHE["nc"] = nc
    return nc


def _sigmoid(x):
    return 1.0 / (1.0 + np.exp(-x))


def host_prep(inputs):
    """Exact host-side weight folds/layout. Returns the shared in_map dict."""
    f32 = np.float32
    ns1 = np.asarray(inputs["norm1_scale"], f32)
    ns2 = np.asarray(inputs["norm2_scale"], f32)
    gate = f32(_sigmoid(np.float64(np.asarray(inputs["gate_logit"]))))
    alpha = f32(_sigmoid(np.float64(np.asarray(inputs["alpha_logit"]))))
    gamma = (GAMMA_MIN + (GAMMA_MAX - GAMMA_MIN)
             * _sigmoid(np.asarray(inputs["decay_logit"], np.float64))).astype(f32)

    KT = np.ascontiguousarray((gate * np.asarray(inputs["k_base"], f32)).T)
    v_eff = np.ascontiguousarray(ns1[:, None] * np.asarray(inputs["v"], f32))
    u_effT = np.ascontiguousarray(
        (alpha * np.asarray(inputs["u"], f32) / ns1[:, None]).T)
    pw_lhsT = (np.asarray(inputs["proj_w"], f32) * ns1[None, :]).T
    up_lhsT = (np.asarray(inputs["up_w"], f32) * ns2[None, :]).T
    dw_lhsT = np.asarray(inputs["down_w"], f32).T

    # block layouts: out-chunk-major [nout, 128(contract sub), nin*128]
    pw = np.ascontiguousarray(
        pw_lhsT.reshape(ND, 128, ND, 128).transpose(2, 1, 0, 3).reshape(ND, 128, D))
    up = np.ascontiguousarray(
        up_lhsT.reshape(ND, 128, NF, 128).transpose(2, 1, 0, 3).reshape(NF, 128, D))
    dw = np.ascontiguousarray(
        dw_lhsT.reshape(NF, 128, ND, 128).transpose(2, 1, 0, 3).reshape(ND, 128, F))

    return dict(
        KT=KT, v_eff=v_eff, u_effT=u_effT, pw=pw, up=up, dw=dw,
        gamma_t=np.ascontiguousarray(np.repeat(gamma[:, None], W, axis=1)),
        projb=np.ascontiguousarray(
            np.asarray(inputs["proj_b"], f32).reshape(ND, 128).T),
        downb=np.ascontiguousarray(
            np.asarray(inputs["down_b"], f32).reshape(ND, 128).T),
        upb=np.ascontiguousarray(
            np.asarray(inputs["up_b"], f32).reshape(NF, 128).T),
        ident=np.eye(128, dtype=f32),
    )


def make_in_maps(inputs):
    shared = host_prep(inputs)
    h = np.asarray(inputs["h"], np.float32)
    in_maps = []
    for b in range(B):
        m = dict(shared)
        m["h_tok"] = np.ascontiguousarray(h[b])
        m["h_T"] = np.ascontiguousarray(h[b].T)
        in_maps.append(m)
    return in_maps


def kernel(**inputs):
    nc = _build()
    in_maps = make_in_maps(inputs)
    res = run_bass_kernel_spmd(nc, in_maps, list(range(B)))
    out = np.stack([np.asarray(res.results[i]["yT"]).T for i in range(B)])
    return np.ascontiguousarray(out.astype(np.float32))



# revision 6
# speedup vs baseline: 1.5618x; 1.5618x over previous
"""Trainium2 Bass kernel for nn_KStackModel (sparse_attention).

Strategy: data-parallel over batch (8 batches -> 8 cores, no collectives).
All heavy matmuls run in bf16 (1 cyc/row on the PE vs 4 for fp32); fp32 is
kept on the residual/stats path so the output error stays ~1e-3.

Per core, feature-major activation layout (tokens on the free axis):

  hT [d,t] fp32 is the only per-core tensor DMA'd in.
  rms1 stats: Square (ACT, bf16 out) -> PE ones-column reduce -> rstd row.
  hb16 = bf16(hT); token-major hn blocks come from PE transposes of hb16,
  with rstd folded in at PSUM eviction (per-partition ACT scale).
  xv^T[r,t] = v_eff.T @ hb16, scaled by rstd afterwards (commutes).
  mixed^T = DVE tensor_tensor_scan (state = gamma*state + xv).
  out^T[d,t] = sum_s hn[s,d].T @ KT[s,t] over causal 512-blocks (bf16).
  h1^T = pw-blocks.T @ out^T + pu.T @ mixed^T + proj_b + hT  (fp32 kept;
         pu = alpha*(proj_w@u) folds the low-rank output through proj).
  rms2 like rms1; rstd2 scales the up-PSUM before gelu (commutes).
  g^T = gelu(up.T @ h1b + up_b); y^T = dw.T @ g^T + down_b + h1^T.
  y^T DMA'd out fp32; host transposes back.

All weight folds (norm scales, gate, alpha, proj@u) are exact host-side
algebra; weights ship as bf16, halving both PCIe and HBM traffic.
"""
import numpy as np
from contextlib import ExitStack

import concourse.bass as bass
import concourse.bacc as bacc
import concourse.tile as tile
from concourse import mybir
from concourse.bass_utils import run_bass_kernel_spmd
import ml_dtypes

B, W, D, R, F = 8, 1024, 1024, 32, 2048
NT, ND, NF = W // 128, D // 128, F // 128   # 8, 8, 16
FP = mybir.dt.float32
BF = mybir.dt.bfloat16
GAMMA_MIN, GAMMA_MAX = 0.15, 1.0
AF = mybir.ActivationFunctionType
ALU = mybir.AluOpType
BFNP = ml_dtypes.bfloat16


def _emit(ctx, tc, a):
    nc = tc.nc

    con = ctx.enter_context(tc.tile_pool(name="con", bufs=1))
    hpool = ctx.enter_context(tc.tile_pool(name="hpool", bufs=9))
    hbf = ctx.enter_context(tc.tile_pool(name="hbf", bufs=8))
    htk = ctx.enter_context(tc.tile_pool(name="htk", bufs=8))
    sqp = ctx.enter_context(tc.tile_pool(name="sqp", bufs=2))
    ktp = ctx.enter_context(tc.tile_pool(name="ktp", bufs=8))
    outp = ctx.enter_context(tc.tile_pool(name="outp", bufs=8))
    h1bp = ctx.enter_context(tc.tile_pool(name="h1bp", bufs=8))
    gpool = ctx.enter_context(tc.tile_pool(name="gpool", bufs=16))
    pwp = ctx.enter_context(tc.tile_pool(name="pwp", bufs=2))
    upp = ctx.enter_context(tc.tile_pool(name="upp", bufs=4))
    dwp = ctx.enter_context(tc.tile_pool(name="dwp", bufs=2))
    yst = ctx.enter_context(tc.tile_pool(name="yst", bufs=2))
    rows = ctx.enter_context(tc.tile_pool(name="rows", bufs=2))
    r32 = ctx.enter_context(tc.tile_pool(name="r32", bufs=3))
    pmm = ctx.enter_context(tc.tile_pool(name="pmm", bufs=2, space="PSUM"))
    ptp = ctx.enter_context(tc.tile_pool(name="ptp", bufs=2, space="PSUM"))
    psm = ctx.enter_context(tc.tile_pool(name="psm", bufs=1, space="PSUM"))

    # ---- constants ----
    zeros_c = con.tile([128, 1], FP, tag="zeros_c")
    nc.vector.memset(zeros_c[:], 0.0)
    nc.const_aps.aps[(FP, 0.0)] = zeros_c[:]
    eps_c = con.tile([128, 1], FP, tag="eps_c")
    nc.vector.memset(eps_c[:], 1e-8)
    nc.const_aps.aps[(FP, 1e-8)] = eps_c[:]
    ones_cb = con.tile([128, 1], BF, tag="ones_cb")
    nc.vector.memset(ones_cb[:], 1.0)
    ones_f = con.tile([128, 1], FP, tag="ones_f")
    nc.vector.memset(ones_f[:], 1.0)
    ones_r128 = con.tile([1, 128], FP, tag="ones_r128")
    nc.vector.memset(ones_r128[:], 1.0)
    eyeb = con.tile([128, 128], BF, tag="eyeb")
    nc.sync.dma_start(eyeb[:], a["eyeb"][:, :])
    gam_sb = con.tile([R, W], FP, tag="gam_sb")
    nc.sync.dma_start(gam_sb[:], a["gamma_t"][:, :])
    pu_sb = con.tile([R, D], BF, tag="pu_sb")
    nc.sync.dma_start(pu_sb[:], a["puT"][:, :])
    projb = con.tile([128, ND], FP, tag="projb")
    nc.sync.dma_start(projb[:], a["projb"][:, :])
    downb = con.tile([128, ND], FP, tag="downb")
    nc.sync.dma_start(downb[:], a["downb"][:, :])
    upb = con.tile([128, NF], FP, tag="upb")
    nc.sync.dma_start(upb[:], a["upb"][:, :])
    v_sb = []
    for dj in range(ND):
        t = con.tile([128, R], BF, tag=f"v{dj}")
        nc.sync.dma_start(t[:], a["v_eff"][dj * 128:(dj + 1) * 128, :])
        v_sb.append(t)

    # ---- load hT; rms1 stats; bf16 copy ----
    hT, sq = [], []
    for dj in range(ND):
        ht = hpool.tile([128, W], FP, tag="h")
        nc.sync.dma_start(ht[:], a["hT"][dj * 128:(dj + 1) * 128, :])
        hT.append(ht)
    hb = []
    for dj in range(ND):
        s = sqp.tile([128, W], BF, tag="sq")
        nc.scalar.activation(s[:], hT[dj][:], AF.Square)
        sq.append(s)
        b = hbf.tile([128, W], BF, tag="hb")
        nc.vector.tensor_copy(b[:], hT[dj][:])
        hb.append(b)
    p_ssq = psm.tile([1, W], FP, tag="pssq", bufs=1)
    for dj in range(ND):
        for c in range(2):
            nc.tensor.matmul(p_ssq[0:1, c * 512:(c + 1) * 512], ones_cb[:],
                             sq[dj][:, c * 512:(c + 1) * 512],
                             start=(dj == 0), stop=(dj == ND - 1))
    std_row = rows.tile([1, W], FP, tag="row")
    nc.scalar.activation(std_row[:], p_ssq[:], AF.Sqrt, bias=1e-8, scale=1.0 / D)
    rstd_row = rows.tile([1, W], FP, tag="row")
    nc.vector.reciprocal(rstd_row[:], std_row[:])

    # rstd as per-partition columns: rstd_col[:, ti] = rstd_row[0, ti*128:...]
    # via plain matmuls: lhsT = rstd chunk [1,128] (stationary), rhs = ones [1,1]
    p_col = psm.tile([128, NT], FP, tag="pcol")
    for ti in range(NT):
        nc.tensor.matmul(p_col[:, ti:ti + 1], rstd_row[0:1, ti * 128:(ti + 1) * 128],
                         ones_f[0:1, 0:1], start=True, stop=True)
    rstd_col = con.tile([128, NT], FP, tag="rstd_col")
    nc.vector.tensor_copy(rstd_col[:], p_col[:])

    # ---- token-major hn blocks: PE-transpose hb16, scale by rstd at eviction ----
    htok = []
    for sj in range(NT):
        t = htk.tile([128, D], BF, tag="htok")
        for half in range(2):
            pt = ptp.tile([128, 512], BF, tag="ptp")
            for k in range(4):
                dj = half * 4 + k
                nc.tensor.transpose(pt[:, k * 128:(k + 1) * 128],
                                    hb[dj][:, sj * 128:(sj + 1) * 128], eyeb[:])
            nc.scalar.activation(t[:, half * 512:(half + 1) * 512], pt[:],
                                 AF.Copy, scale=rstd_col[:, sj:sj + 1])
        htok.append(t)

    # ---- xv^T [R, W] = v_eff.T @ h (raw), then * rstd ----
    xv_raw = r32.tile([R, W], FP, tag="r32")
    for c in range(2):
        pxv = psm.tile([R, 512], FP, tag="pxv", bufs=1)
        for dj in range(ND):
            nc.tensor.matmul(pxv[:], v_sb[dj][:], hb[dj][:, c * 512:(c + 1) * 512],
                             start=(dj == 0), stop=(dj == ND - 1))
        nc.vector.tensor_copy(xv_raw[:, c * 512:(c + 1) * 512], pxv[:])
    rep32 = r32.tile([R, W], FP, tag="r32")
    for c in range(2):
        prep = psm.tile([R, 512], FP, tag="pxv", bufs=1)
        nc.tensor.matmul(prep[:], ones_r128[0:1, 0:R],
                         rstd_row[0:1, c * 512:(c + 1) * 512], start=True, stop=True)
        nc.vector.tensor_copy(rep32[:, c * 512:(c + 1) * 512], prep[:])
    xvT = r32.tile([R, W], FP, tag="r32")
    nc.vector.tensor_mul(xvT[:], xv_raw[:], rep32[:])

    # ---- decay scan; bf16 copy for the pu matmul ----
    mixedT = r32.tile([R, W], FP, tag="r32")
    nc.vector.tensor_tensor_scan(mixedT[:], gam_sb[:], xvT[:], 0.0, ALU.mult, ALU.add)
    mixedb = con.tile([R, W], BF, tag="mixedb")
    nc.vector.tensor_copy(mixedb[:], mixedT[:])

    # ---- base mixing: out^T[d,t] = sum_s hn[s,d].T @ KT[s,t] (causal blocks) ----
    outT = [outp.tile([128, W], BF, tag="outT", name=f"outT{dj}")
            for dj in range(ND)]
    for tcc in range(2):
        sjs = list(range(4)) if tcc == 0 else list(range(8))
        kts = {}
        for sj in sjs:
            kt = ktp.tile([128, 512], BF, tag="kt")
            nc.sync.dma_start(kt[:], a["KT"][sj * 128:(sj + 1) * 128,
                                             tcc * 512:(tcc + 1) * 512])
            kts[sj] = kt
        for dj in range(ND):
            po = pmm.tile([128, 512], FP, tag="pmm")
            for i, sj in enumerate(sjs):
                nc.tensor.matmul(po[:], htok[sj][:, dj * 128:(dj + 1) * 128], kts[sj][:],
                                 start=(i == 0), stop=(i == len(sjs) - 1))
            nc.vector.tensor_copy(outT[dj][:, tcc * 512:(tcc + 1) * 512], po[:])

    # ---- h1^T = pw.T @ out^T + pu.T @ mixed^T + proj_b + hT (fp32 + bf16 copy) ----
    h1T, h1b = [], []
    for dj2 in range(ND):
        pw_t = pwp.tile([128, D], BF, tag="pw")
        nc.sync.dma_start(pw_t[:], a["pw"][dj2, :, :])
        h1 = hpool.tile([128, W], FP, tag="h")
        b1 = h1bp.tile([128, W], BF, tag="h1b")
        for tcc in range(2):
            ph = pmm.tile([128, 512], FP, tag="pmm")
            for dj in range(ND):
                nc.tensor.matmul(ph[:], pw_t[:, dj * 128:(dj + 1) * 128],
                                 outT[dj][:, tcc * 512:(tcc + 1) * 512],
                                 start=(dj == 0), stop=False)
            nc.tensor.matmul(ph[:], pu_sb[:, dj2 * 128:(dj2 + 1) * 128],
                             mixedb[:, tcc * 512:(tcc + 1) * 512],
                             start=False, stop=True)
            sl = slice(tcc * 512, (tcc + 1) * 512)
            nc.scalar.activation(h1[:, sl], ph[:], AF.Identity,
                                 bias=projb[:, dj2:dj2 + 1], scale=1.0)
            nc.vector.tensor_add(h1[:, sl], h1[:, sl], hT[dj2][:, sl])
            nc.scalar.activation(b1[:, sl], h1[:, sl], AF.Copy)
        h1T.append(h1)
        h1b.append(b1)

    # ---- rms2 ----
    p_ssq2 = psm.tile([1, W], FP, tag="pssq", bufs=1)
    for dj2 in range(ND):
        s2 = sqp.tile([128, W], BF, tag="sq")
        nc.scalar.activation(s2[:], h1T[dj2][:], AF.Square)
        for c in range(2):
            nc.tensor.matmul(p_ssq2[0:1, c * 512:(c + 1) * 512], ones_cb[:],
                             s2[:, c * 512:(c + 1) * 512],
                             start=(dj2 == 0), stop=(dj2 == ND - 1))
    std2 = rows.tile([1, W], FP, tag="row")
    nc.scalar.activation(std2[:], p_ssq2[:], AF.Sqrt, bias=1e-8, scale=1.0 / D)
    rstd2_row = rows.tile([1, W], FP, tag="row")
    nc.vector.reciprocal(rstd2_row[:], std2[:])
    rep2 = con.tile([128, W], FP, tag="rep2")
    for c in range(2):
        pr2 = pmm.tile([128, 512], FP, tag="pmm")
        nc.tensor.matmul(pr2[:], ones_r128[:],
                         rstd2_row[0:1, c * 512:(c + 1) * 512], start=True, stop=True)
        nc.vector.tensor_copy(rep2[:, c * 512:(c + 1) * 512], pr2[:])

    # ---- up + gelu: g = gelu((up.T @ h1b) * rstd2 + up_b) ----
    gT = []
    for fi in range(NF):
        up_t = upp.tile([128, D], BF, tag="up")
        nc.sync.dma_start(up_t[:], a["up"][fi, :, :])
        g = gpool.tile([128, W], BF, tag="g")
        for tcc in range(2):
            pg = pmm.tile([128, 512], FP, tag="pmm")
            for dj in range(ND):
                nc.tensor.matmul(pg[:], up_t[:, dj * 128:(dj + 1) * 128],
                                 h1b[dj][:, tcc * 512:(tcc + 1) * 512],
                                 start=(dj == 0), stop=(dj == ND - 1))
            nc.vector.tensor_mul(pg[:], pg[:], rep2[:, tcc * 512:(tcc + 1) * 512])
            nc.scalar.activation(g[:, tcc * 512:(tcc + 1) * 512], pg[:],
                                 AF.Gelu_apprx_tanh, bias=upb[:, fi:fi + 1], scale=1.0)
        gT.append(g)

    # ---- down + residual: y^T = dw.T @ g^T + down_b + h1^T ----
    for dj2 in range(ND):
        dw_t = dwp.tile([128, F], BF, tag="dw")
        nc.sync.dma_start(dw_t[:], a["dw"][dj2, :, :])
        for tcc in range(2):
            py = pmm.tile([128, 512], FP, tag="pmm")
            for fi in range(NF):
                nc.tensor.matmul(py[:], dw_t[:, fi * 128:(fi + 1) * 128],
                                 gT[fi][:, tcc * 512:(tcc + 1) * 512],
                                 start=(fi == 0), stop=(fi == NF - 1))
            y = yst.tile([128, 512], FP, tag="yst")
            nc.scalar.activation(y[:], py[:], AF.Identity,
                                 bias=downb[:, dj2:dj2 + 1], scale=1.0)
            sl = slice(tcc * 512, (tcc + 1) * 512)
            nc.vector.tensor_add(y[:], y[:], h1T[dj2][:, sl])
            nc.sync.dma_start(a["yT"][dj2 * 128:(dj2 + 1) * 128, sl], y[:])


_NC_CACHE = {}


def _build():
    if "nc" in _NC_CACHE:
        return _NC_CACHE["nc"]
    nc = bacc.Bacc("TRN2", target_bir_lowering=False, debug=False)

    def P(name, shape, dt=FP, out=False):
        return nc.declare_dram_parameter(name, list(shape), dt, isOutput=out)

    a = dict(
        hT=P("hT", (D, W)),
        KT=P("KT", (W, W), BF),
        v_eff=P("v_eff", (D, R), BF),
        puT=P("puT", (R, D), BF),
        pw=P("pw", (ND, 128, D), BF),
        up=P("up", (NF, 128, D), BF),
        dw=P("dw", (ND, 128, F), BF),
        gamma_t=P("gamma_t", (R, W)),
        projb=P("projb", (128, ND)),
        downb=P("downb", (128, ND)),
        upb=P("upb", (128, NF)),
        eyeb=P("eyeb", (128, 128), BF),
        yT=P("yT", (D, W), out=True),
    )
    with ExitStack() as ctx:
        tcx = ctx.enter_context(tile.TileContext(nc))
        _emit(ctx, tcx, a)
    nc.finalize()
    _NC_CACHE["nc"] = nc
    return nc


def _sigmoid(x):
    return 1.0 / (1.0 + np.exp(-x))


def host_prep(inputs):
    """Exact host-side weight folds/layout. Returns the shared in_map dict."""
    f32 = np.float32
    ns1 = np.asarray(inputs["norm1_scale"], f32)
    ns2 = np.asarray(inputs["norm2_scale"], f32)
    gate = f32(_sigmoid(np.float64(np.asarray(inputs["gate_logit"]))))
    alpha = f32(_sigmoid(np.float64(np.asarray(inputs["alpha_logit"]))))
    gamma = (GAMMA_MIN + (GAMMA_MAX - GAMMA_MIN)
             * _sigmoid(np.asarray(inputs["decay_logit"], np.float64))).astype(f32)

    kb = np.asarray(inputs["k_base"], f32) * np.tril(np.ones((W, W), f32))
    KT = np.ascontiguousarray((gate * kb).T).astype(BFNP)
    v_eff = (ns1[:, None] * np.asarray(inputs["v"], f32)).astype(BFNP)
    proj_w = np.asarray(inputs["proj_w"], f32)
    puT = np.ascontiguousarray(
        (alpha * (proj_w @ np.asarray(inputs["u"], f32))).T).astype(BFNP)
    pw_lhsT = (proj_w * ns1[None, :]).T
    up_lhsT = (np.asarray(inputs["up_w"], f32) * ns2[None, :]).T
    dw_lhsT = np.asarray(inputs["down_w"], f32).T

    # block layouts: out-chunk-major [nout, 128(contract sub), nin*128]
    pw = np.ascontiguousarray(
        pw_lhsT.reshape(ND, 128, ND, 128).transpose(2, 1, 0, 3).reshape(ND, 128, D)
    ).astype(BFNP)
    up = np.ascontiguousarray(
        up_lhsT.reshape(ND, 128, NF, 128).transpose(2, 1, 0, 3).reshape(NF, 128, D)
    ).astype(BFNP)
    dw = np.ascontiguousarray(
        dw_lhsT.reshape(NF, 128, ND, 128).transpose(2, 1, 0, 3).reshape(ND, 128, F)
    ).astype(BFNP)

    return dict(
        KT=KT, v_eff=v_eff, puT=puT, pw=pw, up=up, dw=dw,
        gamma_t=np.ascontiguousarray(np.repeat(gamma[:, None], W, axis=1)),
        projb=np.ascontiguousarray(
            np.asarray(inputs["proj_b"], f32).reshape(ND, 128).T),
        downb=np.ascontiguousarray(
            np.asarray(inputs["down_b"], f32).reshape(ND, 128).T),
        upb=np.ascontiguousarray(
            np.asarray(inputs["up_b"], f32).reshape(NF, 128).T),
        eyeb=np.eye(128, dtype=np.float32).astype(BFNP),
    )


def make_in_maps(inputs):
    shared = host_prep(inputs)
    h = np.asarray(inputs["h"], np.float32)
    in_maps = []
    for b in range(B):
        m = dict(shared)
        m["hT"] = np.ascontiguousarray(h[b].T)
        in_maps.append(m)
    return in_maps


def kernel(**inputs):
    nc = _build()
    in_maps = make_in_maps(inputs)
    res = run_bass_kernel_spmd(nc, in_maps, list(range(B)))
    out = np.stack([np.asarray(res.results[i]["yT"]).T for i in range(B)])
    return np.ascontiguousarray(out.astype(np.float32))


# revision 16
# speedup vs baseline: 1.8958x; 1.2139x over previous
"""Trainium2 Bass kernel for nn_KStackModel (sparse_attention).

Strategy: data-parallel over batch (8 batches -> 8 cores, no collectives).
Heavy matmuls run bf16 (1 cyc/row on the PE vs 4 for fp32); the MLP up/down
matmuls run fp8e4 in DoubleRow perf mode (0.5 cyc/row, K=256/instr).

Per core (feature-major activations, tokens on the free axis):

  h ships twice, pre-swizzled on the host into single-DMA layouts:
  h_tok [t,d] bf16 (2 halves) and hTb [d,t] bf16. DMA dispatch is ~650ns
  each on the sync queue, so everything ships in ~10 large transfers.
  rms1: ACT Square+accum_out on h_tok -> rstd_col [128,8], in two halves so
  the base matmuls start after the first 4 token tiles; hn = h_tok *
  rstd_col in place (DVE per-partition scale). rstd_row via PE transposes.
  xv^T[r,t] = v_eff.T @ hTb, scaled by rstd_row after (commutes);
  mixed^T = DVE tensor_tensor_scan (state = gamma*state + xv).
  out^T[d,t] = sum_s hn[s,d].T @ KT[s,t] over causal 512-blocks (bf16),
  evicted to bf16 on the (otherwise idle) GPSIMD engine.
  h1^T = pw.T @ out^T + pu.T @ mixed^T + proj_b + hTb (fp32 accum; pu =
  alpha*proj_w@u folds the low-rank output through proj); rms2 stats
  interleaved (squares on GPSIMD, ones-column reduce on the PE).
  h2 = h1 * rstd2 -> fp8 pair tiles [128,2,W] (DoubleRow rhs layout).
  g8 = fp8(gelu((8*up).T @ h2 / 8 + up_b))   (weights pre-scaled x8 on the
  host to dodge fp8 subnormals; /8 exact via the ACT scale operand).
  y^T = (16*dw).T @ g8 / 16 + down_b + h1^T; DMA out fp32, host transposes.

All weight folds (norm scales, gate, alpha, proj@u, fp8 scaling) are exact
host-side algebra; weights ship bf16/fp8, cutting PCIe and HBM traffic.
"""
import numpy as np
from contextlib import ExitStack

import concourse.bass as bass
import concourse.bacc as bacc
import concourse.tile as tile
from concourse import mybir
from concourse.bass_utils import run_bass_kernel_spmd
import ml_dtypes

B, W, D, R, F = 8, 1024, 1024, 32, 2048
NT, ND, NF = W // 128, D // 128, F // 128   # 8, 8, 16
FP = mybir.dt.float32
BF = mybir.dt.bfloat16
F8 = mybir.dt.float8e4
GAMMA_MIN, GAMMA_MAX = 0.15, 1.0
AF = mybir.ActivationFunctionType
ALU = mybir.AluOpType
PM = mybir.MatmulPerfMode
BFNP = ml_dtypes.bfloat16
F8NP = ml_dtypes.float8_e4m3
UP_SCALE = 8.0
DW_SCALE = 16.0
# (sj, tcc) block order of the packed causal KT blocks
KT_BLOCKS = [(sj, 0) for sj in range(4)] + [(sj, 1) for sj in range(8)]


def _emit(ctx, tc, a):
    nc = tc.nc

    con = ctx.enter_context(tc.tile_pool(name="con", bufs=1))
    h1p = ctx.enter_context(tc.tile_pool(name="h1p", bufs=8))
    htkp = ctx.enter_context(tc.tile_pool(name="htkp", bufs=2))
    hbp = ctx.enter_context(tc.tile_pool(name="hbp", bufs=1))
    sqp = ctx.enter_context(tc.tile_pool(name="sqp", bufs=1))
    sq2p = ctx.enter_context(tc.tile_pool(name="sq2p", bufs=2))
    wp = ctx.enter_context(tc.tile_pool(name="wp", bufs=1))
    outp = ctx.enter_context(tc.tile_pool(name="outp", bufs=8))
    h28p = ctx.enter_context(tc.tile_pool(name="h28p", bufs=4))
    g8p = ctx.enter_context(tc.tile_pool(name="g8p", bufs=8))
    yst = ctx.enter_context(tc.tile_pool(name="yst", bufs=3))
    rows = ctx.enter_context(tc.tile_pool(name="rows", bufs=2))
    r32 = ctx.enter_context(tc.tile_pool(name="r32", bufs=3))
    pmm = ctx.enter_context(tc.tile_pool(name="pmm", bufs=5, space="PSUM"))
    psm = ctx.enter_context(tc.tile_pool(name="psm", bufs=1, space="PSUM"))

    # ---- DMA queue: h_tok h0, KT0, h_tok h1, consts, hTb, KT1, pw, up8, dw8 ----
    htok_t = [htkp.tile([128, 4, D], BF, tag="htok", name=f"htok{hf}")
              for hf in range(2)]
    nc.sync.dma_start(htok_t[0][:], a["h_tok"][0, :, :, :])
    kt0 = con.tile([128, 4, 512], BF, tag="kt0")
    nc.sync.dma_start(kt0[:], a["KT0p"][:, :, :])
    nc.sync.dma_start(htok_t[1][:], a["h_tok"][1, :, :, :])

    def htok(ti):
        return htok_t[ti // 4][:, ti % 4, :]

    # packed fp32 consts: eyef | projb | downb | upb
    cpf = con.tile([128, 128 + 2 * ND + NF], FP, tag="cpf")
    nc.sync.dma_start(cpf[:], a["cpf"][:, :])
    eyef = cpf[:, 0:128]
    projb = cpf[:, 128:128 + ND]
    downb = cpf[:, 128 + ND:128 + 2 * ND]
    upb = cpf[:, 128 + 2 * ND:128 + 2 * ND + NF]
    cpb = con.tile([128, ND * R], BF, tag="cpb")
    nc.sync.dma_start(cpb[:], a["cpb"][:, :])

    def v_sb(dj):
        return cpb[:, dj * R:(dj + 1) * R]

    gam_c = con.tile([R, 1], FP, tag="gam_c")
    nc.sync.dma_start(gam_c[:], a["gamma_t"][:, :])
    pu_sb = con.tile([R, D], BF, tag="pu_sb")
    nc.sync.dma_start(pu_sb[:], a["puT"][:, :])

    # hTb [128, 8, W] bf16 (feature-major h: xv moving operand + residual)
    hbt = hbp.tile([128, ND, W], BF, tag="hb")
    nc.sync.dma_start(hbt[:], a["hTb"][:, :, :])

    def hb(dj):
        return hbt[:, dj, :]

    kt1 = con.tile([128, 8, 512], BF, tag="kt1")
    nc.sync.dma_start(kt1[:], a["KT1p"][:, :, :])

    def kts(sj, tcc):
        return kt0[:, sj, :] if tcc == 0 else kt1[:, sj, :]

    pw_t = wp.tile([128, ND, D], BF, tag="pw")
    nc.sync.dma_start(pw_t[:], a["pw"][:, :, :])
    up_t = wp.tile([128, NF, 4, 2, 128], F8, tag="up8")
    nc.sync.dma_start(up_t[:], a["up8"][:, :, :, :, :])
    dw_t = wp.tile([128, ND, 8, 2, 128], F8, tag="dw8")
    nc.sync.dma_start(dw_t[:], a["dw8"][:, :, :, :, :])

    # ---- const-ap registrations (memsets, no DMA) ----
    zeros_c = con.tile([128, 1], FP, tag="zeros_c")
    nc.vector.memset(zeros_c[:], 0.0)
    nc.const_aps.aps[(FP, 0.0)] = zeros_c[:]
    eps_c = con.tile([128, 1], FP, tag="eps_c")
    nc.vector.memset(eps_c[:], 1e-8)
    nc.const_aps.aps[(FP, 1e-8)] = eps_c[:]
    ones_cf = con.tile([128, 1], FP, tag="ones_cf")
    nc.vector.memset(ones_cf[:], 1.0)
    ones_r128 = con.tile([1, 128], FP, tag="ones_r128")
    nc.vector.memset(ones_r128[:], 1.0)
    acc2 = con.tile([128, W], FP, tag="acc2")
    nc.vector.memset(acc2[:], 0.0)

    # ---- rms1 stats (token-major ACT accum), two halves; hn in place ----
    ssq_col = con.tile([128, NT], FP, tag="ssq_col")
    std_col = con.tile([128, NT], FP, tag="std_col")
    rstd_col = con.tile([128, NT], FP, tag="rstd_col")
    for half in range(2):
        hs = slice(half * 4, (half + 1) * 4)
        for ti in range(half * 4, (half + 1) * 4):
            s = sqp.tile([128, D], BF, tag="sq")
            nc.scalar.activation(s[:], htok(ti), AF.Square,
                                 accum_out=ssq_col[:, ti:ti + 1])
        nc.scalar.activation(std_col[:, hs], ssq_col[:, hs], AF.Sqrt,
                             bias=1e-8, scale=1.0 / D)
        nc.vector.reciprocal(rstd_col[:, hs], std_col[:, hs])
        for ti in range(half * 4, (half + 1) * 4):
            nc.vector.tensor_scalar_mul(htok(ti), htok(ti), rstd_col[:, ti:ti + 1])

    # ---- base mixing: out^T[d,t] = sum_s hn[s,d].T @ KT[s,t] ----
    outT = [outp.tile([128, W], BF, tag="outT", name=f"outT{dj}")
            for dj in range(ND)]

    def base_chunk(tcc):
        # per 256-wide t-chunk c (global chunk 2*tcc+cc): s-blocks sj < 2*(c+1)
        for cc in range(2):
            c = 2 * tcc + cc
            sjs = list(range(min(2 * (c + 1), 8)))
            for dj in range(ND):
                po = pmm.tile([128, 512], FP, tag="pmm")
                for i, sj in enumerate(sjs):
                    nc.tensor.matmul(po[:, 0:256],
                                     htok(sj)[:, dj * 128:(dj + 1) * 128],
                                     kts(sj, tcc)[:, cc * 256:(cc + 1) * 256],
                                     start=(i == 0), stop=(i == len(sjs) - 1))
                nc.vector.tensor_copy(
                    outT[dj][:, c * 256:(c + 1) * 256], po[:, 0:256])

    base_chunk(0)

    # rstd_row [1, W] for the xv scale, via PE transposes of rstd_col
    prow = psm.tile([1, W], FP, tag="prow")
    for ti in range(NT):
        nc.tensor.transpose(prow[0:1, ti * 128:(ti + 1) * 128],
                            rstd_col[:, ti:ti + 1], eyef)
    rstd_row = rows.tile([1, W], FP, tag="row")
    nc.vector.tensor_copy(rstd_row[:], prow[:])

    base_chunk(1)

    # ---- xv^T [R, W] = v_eff.T @ h (raw), then * rstd ----
    xv_raw = r32.tile([R, W], FP, tag="r32")
    for c in range(2):
        pxv = psm.tile([R, 512], FP, tag="pxv", bufs=1)
        for dj in range(ND):
            nc.tensor.matmul(pxv[:], v_sb(dj), hb(dj)[:, c * 512:(c + 1) * 512],
                             start=(dj == 0), stop=(dj == ND - 1))
        nc.vector.tensor_copy(xv_raw[:, c * 512:(c + 1) * 512], pxv[:])
    rep32 = r32.tile([R, W], FP, tag="r32")
    for c in range(2):
        prep = psm.tile([R, 512], FP, tag="pxv", bufs=1)
        nc.tensor.matmul(prep[:], ones_r128[0:1, 0:R],
                         rstd_row[0:1, c * 512:(c + 1) * 512], start=True, stop=True)
        nc.vector.tensor_copy(rep32[:, c * 512:(c + 1) * 512], prep[:])
    xvT = r32.tile([R, W], FP, tag="r32")
    nc.vector.tensor_mul(xvT[:], xv_raw[:], rep32[:])

    # ---- decay scan (gamma broadcast along t); bf16 copy for the pu matmul ----
    mixedT = r32.tile([R, W], FP, tag="r32")
    nc.vector.tensor_tensor_scan(mixedT[:], gam_c[:].to_broadcast((R, W)), xvT[:],
                                 0.0, ALU.mult, ALU.add)
    mixedb = con.tile([R, W], BF, tag="mixedb")
    nc.vector.tensor_copy(mixedb[:], mixedT[:])

    # ---- h1^T = pw.T @ out^T + pu.T @ mixed^T + proj_b + h  (tcc-major);
    #      rms2 stats ride along: squares on GPSIMD, block-sums into acc2 (DVE),
    #      per-chunk rstd2/rep2/h28 overlap the other chunk's matmuls ----
    pssq2 = psm.tile([1, W], FP, tag="prow", bufs=1)
    h1T = [h1p.tile([128, W], FP, tag="h1", name=f"h1_{dj2}") for dj2 in range(ND)]
    h28 = [h28p.tile([128, 2, W], F8, tag="h28", name=f"h28_{m}") for m in range(4)]
    std2 = rows.tile([1, W], FP, tag="row")
    rstd2_row = rows.tile([1, W], FP, tag="row")
    rep2 = con.tile([128, W], FP, tag="rep2")

    def rms2_reduce(c):
        sl = slice(c * 512, (c + 1) * 512)
        nc.tensor.matmul(pssq2[0:1, sl], ones_cf[:], acc2[:, sl],
                         start=True, stop=True)
        nc.scalar.activation(std2[0:1, sl], pssq2[0:1, sl], AF.Sqrt,
                             bias=1e-8, scale=1.0 / D)
        nc.vector.reciprocal(rstd2_row[0:1, sl], std2[0:1, sl])

    def rep2_h28(c):
        sl = slice(c * 512, (c + 1) * 512)
        pr2 = pmm.tile([128, 512], FP, tag="pmm")
        nc.tensor.matmul(pr2[:], ones_r128[:], rstd2_row[0:1, sl],
                         start=True, stop=True)
        nc.vector.tensor_copy(rep2[:, sl], pr2[:])
        for m in range(4):
            for i in range(2):
                nc.vector.tensor_mul(h28[m][:, i, sl], h1T[2 * m + i][:, sl],
                                     rep2[:, sl])

    for tcc in range(2):
        sl = slice(tcc * 512, (tcc + 1) * 512)
        for dj2 in range(ND):
            if tcc == 1 and dj2 == 2:
                rms2_reduce(0)
            if tcc == 1 and dj2 == 5:
                rep2_h28(0)
            ph = pmm.tile([128, 512], FP, tag="pmm")
            for dj in range(ND):
                nc.tensor.matmul(ph[:], pw_t[:, dj2, dj * 128:(dj + 1) * 128],
                                 outT[dj][:, sl],
                                 start=(dj == 0), stop=False)
            nc.tensor.matmul(ph[:], pu_sb[:, dj2 * 128:(dj2 + 1) * 128],
                             mixedb[:, sl], start=False, stop=True)
            h1 = h1T[dj2]
            nc.scalar.activation(h1[:, sl], ph[:], AF.Identity,
                                 bias=projb[:, dj2:dj2 + 1], scale=1.0)
            nc.vector.tensor_add(h1[:, sl], h1[:, sl], hb(dj2)[:, sl])
            s2 = sq2p.tile([128, 512], BF, tag="sq2")
            nc.gpsimd.tensor_mul(s2[:], h1[:, sl], h1[:, sl])
            nc.vector.tensor_add(acc2[:, sl], acc2[:, sl], s2[:])

    # ---- up + gelu then down + residual, tcc-major (fp8 DoubleRow) ----
    g8 = [g8p.tile([128, 2, W], F8, tag="g8", name=f"g8_{m}") for m in range(NF // 2)]
    for tcc in range(2):
        sl = slice(tcc * 512, (tcc + 1) * 512)
        for fi in range(NF):
            pg = pmm.tile([128, 512], FP, tag="pmm")
            for m in range(4):
                nc.tensor.matmul(pg[:], up_t[:, fi, m, :, :], h28[m][:, :, sl],
                                 start=(m == 0), stop=(m == 3),
                                 perf_mode=PM.DoubleRow)
            nc.scalar.activation(g8[fi // 2][:, fi % 2, sl], pg[:],
                                 AF.Gelu_apprx_tanh,
                                 bias=upb[:, fi:fi + 1], scale=1.0 / UP_SCALE)
        for dj2 in range(ND):
            if tcc == 0 and dj2 == 1:
                rms2_reduce(1)
            if tcc == 0 and dj2 == 4:
                rep2_h28(1)
            py = pmm.tile([128, 512], FP, tag="pmm")
            for m in range(8):
                nc.tensor.matmul(py[:], dw_t[:, dj2, m, :, :], g8[m][:, :, sl],
                                 start=(m == 0), stop=(m == 7),
                                 perf_mode=PM.DoubleRow)
            y = yst.tile([128, 512], FP, tag="yst")
            nc.scalar.activation(y[:], py[:], AF.Identity,
                                 bias=downb[:, dj2:dj2 + 1], scale=1.0 / DW_SCALE)
            nc.gpsimd.tensor_add(y[:], y[:], h1T[dj2][:, sl])
            nc.sync.dma_start(a["yT"][dj2, :, sl], y[:])


_NC_CACHE = {}


def _build():
    if "nc" in _NC_CACHE:
        return _NC_CACHE["nc"]
    nc = bacc.Bacc("TRN2", target_bir_lowering=False, debug=False)

    def P(name, shape, dt=FP, out=False):
        return nc.declare_dram_parameter(name, list(shape), dt, isOutput=out)

    a = dict(
        h_tok=P("h_tok", (2, 128, 4, D), BF),
        hTb=P("hTb", (128, ND, W), BF),
        KT0p=P("KT0p", (128, 4, 512), BF),
        KT1p=P("KT1p", (128, 8, 512), BF),
        cpf=P("cpf", (128, 128 + 2 * ND + NF)),
        cpb=P("cpb", (128, ND * R), BF),
        puT=P("puT", (R, D), BF),
        pw=P("pw", (128, ND, D), BF),
        up8=P("up8", (128, NF, 4, 2, 128), F8),
        dw8=P("dw8", (128, ND, 8, 2, 128), F8),
        gamma_t=P("gamma_t", (R, 1)),
        yT=P("yT", (ND, 128, W), out=True),
    )
    with ExitStack() as ctx:
        tcx = ctx.enter_context(tile.TileContext(nc))
        _emit(ctx, tcx, a)
    nc.finalize()
    _NC_CACHE["nc"] = nc
    return nc


def _sigmoid(x):
    return 1.0 / (1.0 + np.exp(-x))


def host_prep(inputs):
    """Exact host-side weight folds/layout. Returns the shared in_map dict."""
    f32 = np.float32
    ns1 = np.asarray(inputs["norm1_scale"], f32)
    ns2 = np.asarray(inputs["norm2_scale"], f32)
    gate = f32(_sigmoid(np.float64(np.asarray(inputs["gate_logit"]))))
    alpha = f32(_sigmoid(np.float64(np.asarray(inputs["alpha_logit"]))))
    gamma = (GAMMA_MIN + (GAMMA_MAX - GAMMA_MIN)
             * _sigmoid(np.asarray(inputs["decay_logit"], np.float64))).astype(f32)

    kb = np.asarray(inputs["k_base"], f32) * np.tril(np.ones((W, W), f32))
    KT = np.ascontiguousarray((gate * kb).T).astype(BFNP)
    KT0p = np.stack([KT[sj * 128:(sj + 1) * 128, 0:512] for sj in range(4)], axis=1)
    KT1p = np.stack([KT[sj * 128:(sj + 1) * 128, 512:1024] for sj in range(8)], axis=1)
    v_eff = (ns1[:, None] * np.asarray(inputs["v"], f32)).astype(BFNP)
    cpb = np.ascontiguousarray(
        v_eff.reshape(ND, 128, R).transpose(1, 0, 2).reshape(128, ND * R))
    proj_w = np.asarray(inputs["proj_w"], f32)
    puT = np.ascontiguousarray(
        (alpha * (proj_w @ np.asarray(inputs["u"], f32))).T).astype(BFNP)
    pw_lhsT = (proj_w * ns1[None, :]).T
    up_lhsT = (np.asarray(inputs["up_w"], f32) * ns2[None, :]).T
    dw_lhsT = np.asarray(inputs["down_w"], f32).T

    # stationary-block layouts, contraction-sub-128 on the partition axis
    pw = np.ascontiguousarray(
        pw_lhsT.reshape(ND, 128, ND, 128).transpose(1, 2, 0, 3).reshape(128, ND, D)
    ).astype(BFNP)
    up8 = np.ascontiguousarray(
        (UP_SCALE * up_lhsT).reshape(4, 2, 128, NF, 128).transpose(2, 3, 0, 1, 4)
    ).astype(F8NP)
    dw8 = np.ascontiguousarray(
        (DW_SCALE * dw_lhsT).reshape(8, 2, 128, ND, 128).transpose(2, 3, 0, 1, 4)
    ).astype(F8NP)

    cpf = np.zeros((128, 128 + 2 * ND + NF), f32)
    cpf[:, 0:128] = np.eye(128, dtype=f32)
    cpf[:, 128:128 + ND] = np.asarray(inputs["proj_b"], f32).reshape(ND, 128).T
    cpf[:, 128 + ND:128 + 2 * ND] = (
        np.asarray(inputs["down_b"], f32).reshape(ND, 128).T)
    cpf[:, 128 + 2 * ND:] = np.asarray(inputs["up_b"], f32).reshape(NF, 128).T

    return dict(
        KT0p=KT0p, KT1p=KT1p, cpf=cpf, cpb=cpb, puT=puT, pw=pw, up8=up8, dw8=dw8,
        gamma_t=np.ascontiguousarray(gamma[:, None]),
    )


def make_in_maps(inputs):
    shared = host_prep(inputs)
    h = np.asarray(inputs["h"], np.float32)
    in_maps = []
    for b in range(B):
        m = dict(shared)
        hb16 = h[b].astype(BFNP)
        m["h_tok"] = np.ascontiguousarray(
            hb16.reshape(2, 4, 128, D).transpose(0, 2, 1, 3))
        m["hTb"] = np.ascontiguousarray(
            hb16.T.reshape(ND, 128, W).transpose(1, 0, 2))
        in_maps.append(m)
    return in_maps


def kernel(**inputs):
    nc = _build()
    in_maps = make_in_maps(inputs)
    res = run_bass_kernel_spmd(nc, in_maps, list(range(B)))
    out = np.stack([np.asarray(res.results[i]["yT"]).reshape(D, W).T
                    for i in range(B)])
    return np.ascontiguousarray(out.astype(np.float32))


# revision 20
# speedup vs baseline: 2.1505x; 1.1344x over previous
"""Trainium2 Bass kernel for nn_KStackModel (sparse_attention).

Strategy: data-parallel over batch (8 batches -> 8 cores, no collectives).
Heavy matmuls run bf16 (1 cyc/row on the PE vs 4 for fp32); the MLP up/down
matmuls run fp8e4 in DoubleRow perf mode (0.5 cyc/row, K=256/instr).

Per core (feature-major activations, tokens on the free axis):

  h ships twice, pre-swizzled on the host into single-DMA layouts:
  h_tok [t,d] bf16 (2 halves) and hTb [d,t] bf16. DMA dispatch is ~650ns
  each on the sync queue, so everything ships in ~10 large transfers.
  rms1: ACT Square+accum_out on h_tok -> rstd_col [128,8], in two halves so
  the base matmuls start after the first 4 token tiles; hn = h_tok *
  rstd_col in place (DVE per-partition scale). rstd_row via PE transposes.
  xv^T[r,t] = v_eff.T @ hTb, scaled by rstd_row after (commutes);
  mixed^T = DVE tensor_tensor_scan (state = gamma*state + xv).
  out^T[d,t] = sum_s hn[s,d].T @ KT[s,t] over causal 512-blocks (bf16),
  evicted to bf16 on the (otherwise idle) GPSIMD engine.
  h1^T = pw.T @ out^T + pu.T @ mixed^T + proj_b + hTb (fp32 accum; pu =
  alpha*proj_w@u folds the low-rank output through proj); rms2 stats
  interleaved (squares on GPSIMD, ones-column reduce on the PE).
  h2 = h1 * rstd2 -> fp8 pair tiles [128,2,W] (DoubleRow rhs layout).
  g8 = fp8(gelu((8*up).T @ h2 / 8 + up_b))   (weights pre-scaled x8 on the
  host to dodge fp8 subnormals; /8 exact via the ACT scale operand).
  y^T = (16*dw).T @ g8 / 16 + down_b + h1^T; DMA out fp32, host transposes.

All weight folds (norm scales, gate, alpha, proj@u, fp8 scaling) are exact
host-side algebra; weights ship bf16/fp8, cutting PCIe and HBM traffic.
"""
import numpy as np
from contextlib import ExitStack

import concourse.bass as bass
import concourse.bacc as bacc
import concourse.tile as tile
from concourse import mybir
from concourse.bass_utils import run_bass_kernel_spmd
import ml_dtypes

B, W, D, R, F = 8, 1024, 1024, 32, 2048
NT, ND, NF = W // 128, D // 128, F // 128   # 8, 8, 16
FP = mybir.dt.float32
BF = mybir.dt.bfloat16
F8 = mybir.dt.float8e4
GAMMA_MIN, GAMMA_MAX = 0.15, 1.0
AF = mybir.ActivationFunctionType
ALU = mybir.AluOpType
PM = mybir.MatmulPerfMode
BFNP = ml_dtypes.bfloat16
F8NP = ml_dtypes.float8_e4m3
UP_SCALE = 8.0
DW_SCALE = 16.0
# (sj, tcc) block order of the packed causal KT blocks
KT_BLOCKS = [(sj, 0) for sj in range(4)] + [(sj, 1) for sj in range(8)]


def _emit(ctx, tc, a):
    nc = tc.nc

    con = ctx.enter_context(tc.tile_pool(name="con", bufs=1))
    h1p = ctx.enter_context(tc.tile_pool(name="h1p", bufs=8))
    htkp = ctx.enter_context(tc.tile_pool(name="htkp", bufs=4))
    hbp = ctx.enter_context(tc.tile_pool(name="hbp", bufs=1))
    sqp = ctx.enter_context(tc.tile_pool(name="sqp", bufs=1))
    sq2p = ctx.enter_context(tc.tile_pool(name="sq2p", bufs=2))
    wp = ctx.enter_context(tc.tile_pool(name="wp", bufs=1))
    outp = ctx.enter_context(tc.tile_pool(name="outp", bufs=8))
    h28p = ctx.enter_context(tc.tile_pool(name="h28p", bufs=4))
    g8p = ctx.enter_context(tc.tile_pool(name="g8p", bufs=8))
    yst = ctx.enter_context(tc.tile_pool(name="yst", bufs=3))
    rows = ctx.enter_context(tc.tile_pool(name="rows", bufs=2))
    r32 = ctx.enter_context(tc.tile_pool(name="r32", bufs=3))
    pmm = ctx.enter_context(tc.tile_pool(name="pmm", bufs=5, space="PSUM"))
    psm = ctx.enter_context(tc.tile_pool(name="psm", bufs=1, space="PSUM"))

    # ---- DMA queue: h_tok h0, KT0, h_tok h1, consts, hTb, KT1, pw, up8, dw8 ----
    htok_t = [htkp.tile([128, 2, D], BF, tag="htok", name=f"htok{hf}")
              for hf in range(4)]
    nc.sync.dma_start(htok_t[0][:], a["h_tok"][0, :, :, :])
    kt0 = con.tile([128, 4, 512], BF, tag="kt0")
    nc.sync.dma_start(kt0[:], a["KT0p"][:, :, :])
    for hf in range(1, 4):
        nc.sync.dma_start(htok_t[hf][:], a["h_tok"][hf, :, :, :])

    def htok(ti):
        return htok_t[ti // 2][:, ti % 2, :]

    # packed fp32 consts: eyef | projb | downb | upb
    cpf = con.tile([128, 128 + 2 * ND + NF], FP, tag="cpf")
    nc.sync.dma_start(cpf[:], a["cpf"][:, :])
    eyef = cpf[:, 0:128]
    projb = cpf[:, 128:128 + ND]
    downb = cpf[:, 128 + ND:128 + 2 * ND]
    upb = cpf[:, 128 + 2 * ND:128 + 2 * ND + NF]
    cpb = con.tile([128, ND * R], BF, tag="cpb")
    nc.sync.dma_start(cpb[:], a["cpb"][:, :])

    def v_sb(dj):
        return cpb[:, dj * R:(dj + 1) * R]

    gam_c = con.tile([R, 1], FP, tag="gam_c")
    nc.sync.dma_start(gam_c[:], a["gamma_t"][:, :])
    pu_sb = con.tile([R, D], BF, tag="pu_sb")
    nc.sync.dma_start(pu_sb[:], a["puT"][:, :])

    # hTb [128, 8, W] bf16 (feature-major h: xv moving operand + residual)
    hbt = hbp.tile([128, ND, W], BF, tag="hb")
    nc.sync.dma_start(hbt[:], a["hTb"][:, :, :])

    def hb(dj):
        return hbt[:, dj, :]

    kt1 = con.tile([128, 8, 512], BF, tag="kt1")
    nc.sync.dma_start(kt1[:], a["KT1p"][:, :, :])

    def kts(sj, tcc):
        return kt0[:, sj, :] if tcc == 0 else kt1[:, sj, :]

    pw_t = wp.tile([128, ND, D], BF, tag="pw")
    nc.sync.dma_start(pw_t[:], a["pw"][:, :, :])
    up_t = wp.tile([128, NF, 4, 2, 128], F8, tag="up8")
    nc.sync.dma_start(up_t[:], a["up8"][:, :, :, :, :])
    dw_t = wp.tile([128, ND, 8, 2, 128], F8, tag="dw8")
    nc.sync.dma_start(dw_t[:], a["dw8"][:, :, :, :, :])

    # ---- const-ap registrations (memsets, no DMA) ----
    zeros_c = con.tile([128, 1], FP, tag="zeros_c")
    nc.vector.memset(zeros_c[:], 0.0)
    nc.const_aps.aps[(FP, 0.0)] = zeros_c[:]
    eps_c = con.tile([128, 1], FP, tag="eps_c")
    nc.vector.memset(eps_c[:], 1e-8)
    nc.const_aps.aps[(FP, 1e-8)] = eps_c[:]
    ones_cf = con.tile([128, 1], FP, tag="ones_cf")
    nc.vector.memset(ones_cf[:], 1.0)
    ones_r128 = con.tile([1, 128], FP, tag="ones_r128")
    nc.vector.memset(ones_r128[:], 1.0)
    acc2 = con.tile([128, W], FP, tag="acc2")
    nc.vector.memset(acc2[:], 0.0)
    # pre-warm the ACT function tables while the first DMAs stream
    # (scratch target: std_col[:, 0:1] is overwritten later by the real Sqrt)

    # ---- rms1 stats (token-major ACT accum), two halves; hn in place ----
    ssq_col = con.tile([128, NT], FP, tag="ssq_col")
    std_col = con.tile([128, NT], FP, tag="std_col")
    rstd_col = con.tile([128, NT], FP, tag="rstd_col")
    nc.scalar.activation(std_col[:, 0:1], zeros_c[:], AF.Square)
    nc.scalar.activation(std_col[:, 0:1], zeros_c[:], AF.Sqrt, bias=1e-8, scale=1.0)
    nc.scalar.activation(std_col[:, 0:1], zeros_c[:], AF.Identity,
                         bias=eps_c[:, 0:1], scale=1.0)
    for half in range(4):
        hs = slice(half * 2, (half + 1) * 2)
        for ti in range(half * 2, (half + 1) * 2):
            s = sqp.tile([128, D], BF, tag="sq")
            nc.scalar.activation(s[:], htok(ti), AF.Square,
                                 accum_out=ssq_col[:, ti:ti + 1])
        nc.scalar.activation(std_col[:, hs], ssq_col[:, hs], AF.Sqrt,
                             bias=1e-8, scale=1.0 / D)
        nc.vector.reciprocal(rstd_col[:, hs], std_col[:, hs])
        for ti in range(half * 2, (half + 1) * 2):
            nc.vector.tensor_scalar_mul(htok(ti), htok(ti), rstd_col[:, ti:ti + 1])

    # ---- base mixing: out^T[d,t] = sum_s hn[s,d].T @ KT[s,t] ----
    outT = [outp.tile([128, W], BF, tag="outT", name=f"outT{dj}")
            for dj in range(ND)]

    def base_chunk(tcc):
        # per 256-wide t-chunk c (global chunk 2*tcc+cc): s-blocks sj < 2*(c+1)
        for cc in range(2):
            c = 2 * tcc + cc
            sjs = list(range(min(2 * (c + 1), 8)))
            for dj in range(ND):
                po = pmm.tile([128, 512], FP, tag="pmm")
                for i, sj in enumerate(sjs):
                    nc.tensor.matmul(po[:, 0:256],
                                     htok(sj)[:, dj * 128:(dj + 1) * 128],
                                     kts(sj, tcc)[:, cc * 256:(cc + 1) * 256],
                                     start=(i == 0), stop=(i == len(sjs) - 1))
                nc.vector.tensor_copy(
                    outT[dj][:, c * 256:(c + 1) * 256], po[:, 0:256])

    base_chunk(0)

    # rstd_row [1, W] for the xv scale, via PE transposes of rstd_col
    prow = psm.tile([1, W], FP, tag="prow")
    for ti in range(NT):
        nc.tensor.transpose(prow[0:1, ti * 128:(ti + 1) * 128],
                            rstd_col[:, ti:ti + 1], eyef)
    rstd_row = rows.tile([1, W], FP, tag="row")
    nc.vector.tensor_copy(rstd_row[:], prow[:])

    base_chunk(1)

    # ---- xv^T [R, W] = v_eff.T @ h (raw), then * rstd ----
    xv_raw = r32.tile([R, W], FP, tag="r32")
    for c in range(2):
        pxv = psm.tile([R, 512], FP, tag="pxv", bufs=1)
        for dj in range(ND):
            nc.tensor.matmul(pxv[:], v_sb(dj), hb(dj)[:, c * 512:(c + 1) * 512],
                             start=(dj == 0), stop=(dj == ND - 1))
        nc.vector.tensor_copy(xv_raw[:, c * 512:(c + 1) * 512], pxv[:])
    rep32 = r32.tile([R, W], FP, tag="r32")
    for c in range(2):
        prep = psm.tile([R, 512], FP, tag="pxv", bufs=1)
        nc.tensor.matmul(prep[:], ones_r128[0:1, 0:R],
                         rstd_row[0:1, c * 512:(c + 1) * 512], start=True, stop=True)
        nc.vector.tensor_copy(rep32[:, c * 512:(c + 1) * 512], prep[:])
    xvT = r32.tile([R, W], FP, tag="r32")
    nc.vector.tensor_mul(xvT[:], xv_raw[:], rep32[:])

    # ---- decay scan (gamma broadcast along t); bf16 copy for the pu matmul ----
    mixedT = r32.tile([R, W], FP, tag="r32")
    nc.vector.tensor_tensor_scan(mixedT[:], gam_c[:].to_broadcast((R, W)), xvT[:],
                                 0.0, ALU.mult, ALU.add)
    mixedb = con.tile([R, W], BF, tag="mixedb")
    nc.vector.tensor_copy(mixedb[:], mixedT[:])

    # ---- h1^T = pw.T @ out^T + pu.T @ mixed^T + proj_b + h  (tcc-major);
    #      rms2 stats ride along: squares on GPSIMD, block-sums into acc2 (DVE),
    #      per-chunk rstd2/rep2/h28 overlap the other chunk's matmuls ----
    pssq2 = psm.tile([1, W], FP, tag="prow", bufs=1)
    h1T = [h1p.tile([128, W], FP, tag="h1", name=f"h1_{dj2}") for dj2 in range(ND)]
    h28 = [h28p.tile([128, 2, W], F8, tag="h28", name=f"h28_{m}") for m in range(4)]
    std2 = rows.tile([1, W], FP, tag="row")
    rstd2_row = rows.tile([1, W], FP, tag="row")
    rep2 = con.tile([128, W], FP, tag="rep2")

    def rms2_reduce(c):
        sl = slice(c * 512, (c + 1) * 512)
        nc.tensor.matmul(pssq2[0:1, sl], ones_cf[:], acc2[:, sl],
                         start=True, stop=True)
        nc.scalar.activation(std2[0:1, sl], pssq2[0:1, sl], AF.Sqrt,
                             bias=1e-8, scale=1.0 / D)
        nc.vector.reciprocal(rstd2_row[0:1, sl], std2[0:1, sl])

    def rep2_h28(c):
        sl = slice(c * 512, (c + 1) * 512)
        pr2 = pmm.tile([128, 512], FP, tag="pmm")
        nc.tensor.matmul(pr2[:], ones_r128[:], rstd2_row[0:1, sl],
                         start=True, stop=True)
        nc.vector.tensor_copy(rep2[:, sl], pr2[:])
        for m in range(4):
            for i in range(2):
                nc.vector.tensor_mul(h28[m][:, i, sl], h1T[2 * m + i][:, sl],
                                     rep2[:, sl])

    for tcc in range(2):
        sl = slice(tcc * 512, (tcc + 1) * 512)
        for dj2 in range(ND):
            if tcc == 1 and dj2 == 3:
                rms2_reduce(0)
            if tcc == 1 and dj2 == 5:
                rep2_h28(0)
            ph = pmm.tile([128, 512], FP, tag="pmm")
            for dj in range(ND):
                nc.tensor.matmul(ph[:], pw_t[:, dj2, dj * 128:(dj + 1) * 128],
                                 outT[dj][:, sl],
                                 start=(dj == 0), stop=False)
            nc.tensor.matmul(ph[:], pu_sb[:, dj2 * 128:(dj2 + 1) * 128],
                             mixedb[:, sl], start=False, stop=True)
            h1 = h1T[dj2]
            nc.scalar.activation(h1[:, sl], ph[:], AF.Identity,
                                 bias=projb[:, dj2:dj2 + 1], scale=1.0)
            nc.vector.tensor_add(h1[:, sl], h1[:, sl], hb(dj2)[:, sl])
            s2 = sq2p.tile([128, 512], BF, tag="sq2")
            nc.gpsimd.tensor_mul(s2[:], h1[:, sl], h1[:, sl])
            nc.vector.tensor_add(acc2[:, sl], acc2[:, sl], s2[:])

    # ---- up + gelu (both chunks), then down + residual (both chunks):
    #      each chunk's gelus drain while the next phase's matmuls run ----
    g8 = [g8p.tile([128, 2, W], F8, tag="g8", name=f"g8_{m}") for m in range(NF // 2)]
    for tcc in range(2):
        sl = slice(tcc * 512, (tcc + 1) * 512)
        for fi in range(NF):
            if tcc == 0 and fi == 6:
                rms2_reduce(1)
            if tcc == 0 and fi == 12:
                rep2_h28(1)
            pg = pmm.tile([128, 512], FP, tag="pmm")
            for m in range(4):
                nc.tensor.matmul(pg[:], up_t[:, fi, m, :, :], h28[m][:, :, sl],
                                 start=(m == 0), stop=(m == 3),
                                 perf_mode=PM.DoubleRow)
            nc.scalar.activation(g8[fi // 2][:, fi % 2, sl], pg[:],
                                 AF.Gelu_apprx_tanh,
                                 bias=upb[:, fi:fi + 1], scale=1.0 / UP_SCALE)
    for tcc in range(2):
        sl = slice(tcc * 512, (tcc + 1) * 512)
        for dj2 in range(ND):
            py = pmm.tile([128, 512], FP, tag="pmm")
            for m in range(8):
                nc.tensor.matmul(py[:], dw_t[:, dj2, m, :, :], g8[m][:, :, sl],
                                 start=(m == 0), stop=(m == 7),
                                 perf_mode=PM.DoubleRow)
            y = yst.tile([128, 512], FP, tag="yst")
            if dj2 % 2 == 0:
                nc.scalar.activation(y[:], py[:], AF.Identity,
                                     bias=downb[:, dj2:dj2 + 1],
                                     scale=1.0 / DW_SCALE)
                nc.gpsimd.tensor_add(y[:], y[:], h1T[dj2][:, sl])
            else:
                nc.vector.tensor_scalar(y[:], py[:], 1.0 / DW_SCALE,
                                        downb[:, dj2:dj2 + 1], ALU.mult, ALU.add)
                nc.vector.tensor_add(y[:], y[:], h1T[dj2][:, sl])
            nc.sync.dma_start(a["yT"][dj2, :, sl], y[:])


_NC_CACHE = {}


def _build():
    if "nc" in _NC_CACHE:
        return _NC_CACHE["nc"]
    nc = bacc.Bacc("TRN2", target_bir_lowering=False, debug=False)

    def P(name, shape, dt=FP, out=False):
        return nc.declare_dram_parameter(name, list(shape), dt, isOutput=out)

    a = dict(
        h_tok=P("h_tok", (4, 128, 2, D), BF),
        hTb=P("hTb", (128, ND, W), BF),
        KT0p=P("KT0p", (128, 4, 512), BF),
        KT1p=P("KT1p", (128, 8, 512), BF),
        cpf=P("cpf", (128, 128 + 2 * ND + NF)),
        cpb=P("cpb", (128, ND * R), BF),
        puT=P("puT", (R, D), BF),
        pw=P("pw", (128, ND, D), BF),
        up8=P("up8", (128, NF, 4, 2, 128), F8),
        dw8=P("dw8", (128, ND, 8, 2, 128), F8),
        gamma_t=P("gamma_t", (R, 1)),
        yT=P("yT", (ND, 128, W), out=True),
    )
    with ExitStack() as ctx:
        tcx = ctx.enter_context(tile.TileContext(nc))
        _emit(ctx, tcx, a)
    nc.finalize()
    _NC_CACHE["nc"] = nc
    return nc


def _sigmoid(x):
    return 1.0 / (1.0 + np.exp(-x))


def host_prep(inputs):
    """Exact host-side weight folds/layout. Returns the shared in_map dict."""
    f32 = np.float32
    ns1 = np.asarray(inputs["norm1_scale"], f32)
    ns2 = np.asarray(inputs["norm2_scale"], f32)
    gate = f32(_sigmoid(np.float64(np.asarray(inputs["gate_logit"]))))
    alpha = f32(_sigmoid(np.float64(np.asarray(inputs["alpha_logit"]))))
    gamma = (GAMMA_MIN + (GAMMA_MAX - GAMMA_MIN)
             * _sigmoid(np.asarray(inputs["decay_logit"], np.float64))).astype(f32)

    kb = np.asarray(inputs["k_base"], f32) * np.tril(np.ones((W, W), f32))
    KT = np.ascontiguousarray((gate * kb).T).astype(BFNP)
    KT0p = np.stack([KT[sj * 128:(sj + 1) * 128, 0:512] for sj in range(4)], axis=1)
    KT1p = np.stack([KT[sj * 128:(sj + 1) * 128, 512:1024] for sj in range(8)], axis=1)
    v_eff = (ns1[:, None] * np.asarray(inputs["v"], f32)).astype(BFNP)
    cpb = np.ascontiguousarray(
        v_eff.reshape(ND, 128, R).transpose(1, 0, 2).reshape(128, ND * R))
    proj_w = np.asarray(inputs["proj_w"], f32)
    puT = np.ascontiguousarray(
        (alpha * (proj_w @ np.asarray(inputs["u"], f32))).T).astype(BFNP)
    pw_lhsT = (proj_w * ns1[None, :]).T
    up_lhsT = (np.asarray(inputs["up_w"], f32) * ns2[None, :]).T
    dw_lhsT = np.asarray(inputs["down_w"], f32).T

    # stationary-block layouts, contraction-sub-128 on the partition axis
    pw = np.ascontiguousarray(
        pw_lhsT.reshape(ND, 128, ND, 128).transpose(1, 2, 0, 3).reshape(128, ND, D)
    ).astype(BFNP)
    up8 = np.ascontiguousarray(
        (UP_SCALE * up_lhsT).reshape(4, 2, 128, NF, 128).transpose(2, 3, 0, 1, 4)
    ).astype(F8NP)
    dw8 = np.ascontiguousarray(
        (DW_SCALE * dw_lhsT).reshape(8, 2, 128, ND, 128).transpose(2, 3, 0, 1, 4)
    ).astype(F8NP)

    cpf = np.zeros((128, 128 + 2 * ND + NF), f32)
    cpf[:, 0:128] = np.eye(128, dtype=f32)
    cpf[:, 128:128 + ND] = np.asarray(inputs["proj_b"], f32).reshape(ND, 128).T
    cpf[:, 128 + ND:128 + 2 * ND] = (
        np.asarray(inputs["down_b"], f32).reshape(ND, 128).T)
    cpf[:, 128 + 2 * ND:] = np.asarray(inputs["up_b"], f32).reshape(NF, 128).T

    return dict(
        KT0p=KT0p, KT1p=KT1p, cpf=cpf, cpb=cpb, puT=puT, pw=pw, up8=up8, dw8=dw8,
        gamma_t=np.ascontiguousarray(gamma[:, None]),
    )


def make_in_maps(inputs):
    shared = host_prep(inputs)
    h = np.asarray(inputs["h"], np.float32)
    in_maps = []
    for b in range(B):
        m = dict(shared)
        hb16 = h[b].astype(BFNP)
        m["h_tok"] = np.ascontiguousarray(
            hb16.reshape(4, 2, 128, D).transpose(0, 2, 1, 3))
        m["hTb"] = np.ascontiguousarray(
            hb16.T.reshape(ND, 128, W).transpose(1, 0, 2))
        in_maps.append(m)
    return in_maps


def kernel(**inputs):
    nc = _build()
    in_maps = make_in_maps(inputs)
    res = run_bass_kernel_spmd(nc, in_maps, list(range(B)))
    out = np.stack([np.asarray(res.results[i]["yT"]).reshape(D, W).T
                    for i in range(B)])
    return np.ascontiguousarray(out.astype(np.float32))
